# revision 56
# baseline (speedup 1.0000x reference)
"""GQA attention with 2D RoPE on 8 TRN2 NeuronCores.

Sharding: batch data-parallel x4  X  head-group tensor-parallel x2.
Core c handles batch b=c//2 and head group g=c%2 (16 Q heads, 4 KV heads).
Each core writes its PARTIAL out-projection y_g = A_g @ woT_g  [896, 2048]
(bf16); the host sums the two partials per batch (no device collective).

All weights/activations enter as bf16 (host-cast).  RoPE cos/sin tables are
host-computed.  Q heads are host-permuted so the 4 Q heads of each KV head
live in the same partition half as that KV head (no K duplication): QT tile
t, slot s  <->  local q head  8*(t//4) + 4*s + t%4, kv head 2*(t//4)+s.

The Q/K/V projections run as 3-term fp8 splits (x = xh+xl, w = wh+wl at a
shared scale, dropping the xl*wl term) with DoubleRow perf mode, which this
PE executes at 4x bf16 throughput; accuracy beats plain bf16.  The 1/512
psum scale folds into the host RoPE tables (q, k) and the Vst ones columns
(denominator), so no extra scaling ops exist on device.

Attention uses 128-wide q blocks, 28 causal (qb, kb) blocks per head in
qb-major order, batched 8-at-a-time so exp runs as 4 big Activation
instructions per head (the Activation engine is the attention bottleneck).
The two slots of each QT tile run as two interleaved streams in a flat
software pipeline where AVs lag scores by 2 batch-pairs; each tile's tail
is flushed early so softmax epilogues overlap the next tile's scores.

V staging per key block kb: Vst[kb] [128 keys, 9, 64] = [1|V0|1|V1|...|V3|1]
(64-col groups).  AV lhsT for kv head k, slot s = cols [128k + 64*(1-s),
+128) which is [V_k|1] for s=0 (numerators rows 0:64, denominator sums rows
64:128) and [1|V_k] for s=1.
"""

import numpy as np
import ml_dtypes

import concourse.bass as bass
import concourse.tile as tile
import concourse.mybir as mybir
from concourse import bacc
from concourse import bass_utils

F32 = mybir.dt.float32
BF16 = mybir.dt.bfloat16
F8 = mybir.dt.float8e4
AF = mybir.ActivationFunctionType
ALU = mybir.AluOpType

B, L, D = 4, 896, 2048
HQ, HKV, HD = 32, 8, 64
NCORES = 8
GO = D // 2          # 1024 q-out dims per core
KVO = HKV * HD // 2  # 256 kv-out dims per core
P = 128
NI = D // P          # 16 contraction chunks
LB = L // P          # 7 key/q blocks

# causal blocks per head, qb-major: [(qb, kb)], kb <= qb
BLOCKS = [(qb, kb) for qb in range(LB) for kb in range(qb + 1)]
SX, SW = 8.0, 64.0   # fp8 operand scales (psum carries SX*SW = 512x)
EXPB = 8             # blocks per exp batch
NBATCH = (len(BLOCKS) + EXPB - 1) // EXPB   # 4 batches: 8,8,8,4

_NC_CACHE = {}


def _ap3(dram_ap, off, ap):
    return bass.AP(tensor=dram_ap.tensor, offset=dram_ap.offset + off, ap=ap)


def _sb(tile_ap, off, ap):
    """Manual sub-AP of an SBUF tile."""
    return bass.AP(tensor=tile_ap.tensor, offset=tile_ap.offset + off, ap=ap)


def build_nc():
    if "nc" in _NC_CACHE:
        return _NC_CACHE["nc"]
    nc = bacc.Bacc("TRN2", target_bir_lowering=False, debug=False,
                   num_devices=NCORES)
    ins = {"woT": nc.dram_tensor("woT", [GO, D], BF16,
                                 kind="ExternalInput").ap(),
           "C": nc.dram_tensor("C", [P, L], BF16, kind="ExternalInput").ap(),
           "S": nc.dram_tensor("S", [P, L], BF16, kind="ExternalInput").ap(),
           "SW": nc.dram_tensor("SW", [P, P], BF16,
                                kind="ExternalInput").ap()}
    for nm, shp in (("xT", [D, L]), ("wqT", [D, GO]), ("wkT", [D, KVO]),
                    ("wvT", [D, KVO])):
        for c in "hl":
            ins[nm + c] = nc.dram_tensor(nm + c, shp, F8,
                                         kind="ExternalInput").ap()
    y = nc.dram_tensor("y", [L, D], BF16, kind="ExternalOutput").ap()
    with tile.TileContext(nc) as tc:
        _build_kernel(nc, tc, ins, y)
    nc.compile()
    _NC_CACHE["nc"] = nc
    return nc


def _rope(nc, pool, raw, C, S, out):
    """out = raw*C + shuffle16(raw)*S   (all [128, L] bf16)."""
    shuf = pool.tile([P, L], BF16, tag="rope_shuf")
    mask = [(p ^ 16) for p in range(32)]
    nc.vector.stream_shuffle(shuf[:], raw[:], mask)
    m1 = pool.tile([P, L], BF16, tag="rope_m1", bufs=1)
    nc.vector.tensor_mul(m1[:], raw[:], C[:])
    m2 = pool.tile([P, L], BF16, tag="rope_m2", bufs=1)
    nc.vector.tensor_mul(m2[:], shuf[:], S[:])
    nc.vector.tensor_add(out[:], m1[:], m2[:])


def _build_kernel(nc, tc, ins, y):
    import contextlib
    ctx = contextlib.ExitStack()
    with ctx:
        const = ctx.enter_context(tc.tile_pool(name="const", bufs=1))
        qt_pool = ctx.enter_context(tc.tile_pool(name="qt", bufs=1))
        kt_pool = ctx.enter_context(tc.tile_pool(name="kt", bufs=1))
        v_pool = ctx.enter_context(tc.tile_pool(name="vx", bufs=1))
        at_pool = ctx.enter_context(tc.tile_pool(name="at", bufs=1))
        wo_pool = ctx.enter_context(tc.tile_pool(name="wo", bufs=1))
        QT = [qt_pool.tile([P, L], BF16, tag=f"qt{t}", name=f"qt{t}")
              for t in range(8)]
        KT = [kt_pool.tile([P, L], BF16, tag=f"kt{i}", name=f"kt{i}")
              for i in range(2)]
        # Vst[kb]: [1|V0|1|V1|1|V2|1|V3|1], 9 64-col groups
        Vst = [v_pool.tile([P, 9, 64], BF16, tag=f"v{b_}", name=f"v{b_}")
               for b_ in range(LB)]
        AT = [at_pool.tile([P, L], BF16, tag=f"at{t}", name=f"at{t}")
              for t in range(8)]
        AT8 = {c: at_pool.tile([P, 8, L], F8, tag=f"at8{c}", name=f"at8{c}")
               for c in "hl"}
        WO8 = {c: wo_pool.tile([P, 8, D], F8, tag=f"wo8{c}", name=f"wo8{c}")
               for c in "hl"}
        Ctab = const.tile([P, L], BF16, tag="Ctab", name="Ctab")
        Stab = const.tile([P, L], BF16, tag="Stab", name="Stab")
        dum_src = const.tile([P, P], BF16, tag="dum_src", name="dum_src")
        nc.vector.memset(dum_src[:], 0.0)
        SWt = const.tile([P, P], BF16, tag="SWt", name="SWt")
        for b_ in range(LB):
            for j in range(5):
                nc.vector.memset(Vst[b_][:, 2 * j, :], SX * SW)

        with tc.tile_pool(name="xt", bufs=1) as xt_pool, \
             tc.tile_pool(name="wst", bufs=1) as wst, \
             tc.tile_pool(name="ev", bufs=2) as ev:
            X8 = {c: xt_pool.tile([P, NI, L], F8, tag=f"x{c}", name=f"x{c}")
                  for c in "hl"}
            WV8 = {c: wst.tile([P, NI, KVO], F8, tag=f"wv{c}", name=f"wv{c}")
                   for c in "hl"}
            WK8 = {c: wst.tile([P, NI, KVO], F8, tag=f"wk{c}", name=f"wk{c}")
                   for c in "hl"}
            WQ8 = [{c: wst.tile([P, NI, 512], F8, tag=f"wq{og}{c}",
                               name=f"wq{og}{c}") for c in "hl"}
                   for og in range(2)]

            def ld_x(c, c0, cn):
                nc.sync.dma_start(
                    X8[c][:, c0:c0 + cn, :],
                    _ap3(ins["xT" + c], c0 * P * L,
                         [[L, P], [P * L, cn], [1, L]]))

            def ld_wq(og, c, c0, cn):
                nc.sync.dma_start(
                    WQ8[og][c][:, c0:c0 + cn, :],
                    _ap3(ins["wqT" + c], og * 512 + c0 * P * GO,
                         [[GO, P], [P * GO, cn], [1, 512]]))

            def ld_wkv(name, dst, c, c0, cn):
                nc.sync.dma_start(
                    dst[c][:, c0:c0 + cn, :],
                    _ap3(ins[name + c], c0 * P * KVO,
                         [[KVO, P], [P * KVO, cn], [1, KVO]]))

            # strict consumption order: Q0 pass0 (wq0h+xh), pass1 (wq0l),
            # pass2 (xl), og1 weights, rope tables, K, V, wo
            for c0 in range(0, 16, 4):
                ld_wq(0, "h", c0, 4); ld_x("h", c0, 4); ld_wq(0, "l", c0, 4)
            for c0 in range(0, 16, 4):
                ld_x("l", c0, 4)
            ld_wq(1, "h", 0, 4); ld_wq(1, "h", 4, 4)
            ld_wq(1, "h", 8, 4); ld_wq(1, "h", 12, 4)
            ld_wq(1, "l", 0, 8); ld_wq(1, "l", 8, 8)
            nc.sync.dma_start(Ctab[:], ins["C"][:, :])
            nc.sync.dma_start(Stab[:], ins["S"][:, :])
            nc.sync.dma_start(SWt[:], ins["SW"][:, :])
            ld_wkv("wkT", WK8, "h", 0, 16); ld_wkv("wkT", WK8, "l", 0, 16)
            ld_wkv("wvT", WV8, "h", 0, 16); ld_wkv("wvT", WV8, "l", 0, 16)
            for c in "hl":
                for h in range(2):
                    nc.sync.dma_start(
                        WO8[c][:, 4 * h:4 * h + 4, :],
                        _ap3(ins["woT" + c], 4 * h * P * D,
                             [[D, P], [P * D, 4], [1, D]]))

            # ---------- phase 1: Q og0, K, V projections -----------------
            with tc.tile_pool(name="ps1", bufs=1, space="PSUM") as ps1:
                dumps = ps1.tile([P, P], F32, tag="ps1_7", name="dumps")

                def dummies(n):
                    # keep the PE continuously busy across a known stall so
                    # the p-state never drops (each is [128,128], ~53 ns)
                    for _ in range(n):
                        nc.tensor.matmul(dumps[:], dum_src[:], dum_src[:],
                                         start=True, stop=True)

                dummies(38)
                # 3-term fp8 split: xh@wh + xh@wl + xl@wh, DoubleRow pairs
                # of 128-chunks (8 dr-chunks of 256 contraction each)
                PASSES = (("h", "h"), ("h", "l"), ("l", "h"))
                psq = [ps1.tile([P, 448], F32, tag=f"ps1_{j}",
                                name=f"psq0_{j}") for j in range(8)]

                def q_mm(og, psum, p, i, ob, h2):
                    xc, wc = PASSES[p]
                    nc.tensor.matmul(
                        psum[ob * 2 + h2][:],
                        WQ8[og][wc][:, 2 * i:2 * i + 2, ob * P:(ob + 1) * P],
                        X8[xc][:, 2 * i:2 * i + 2,
                               h2 * 448:(h2 + 1) * 448],
                        start=(p == 0 and i == 0),
                        stop=(p == 2 and i == NI // 2 - 1),
                        perf_mode=mybir.MatmulPerfMode.DoubleRow)

                # interleave passes 0/1 per chunk: pass0 alone is gated by
                # the wq0h+xh DMA stream; pass1 reuses xh so it fills the gaps
                for i in range(NI // 2):
                    for p in range(2):
                        for ob in range(4):
                            for h2 in range(2):
                                q_mm(0, psq, p, i, ob, h2)
                for i in range(NI // 2):
                    for ob in range(4):
                        for h2 in range(2):
                            q_mm(0, psq, 2, i, ob, h2)
                for ob in range(4):
                    raw = ev.tile([P, L], BF16, tag="qraw")
                    for h2 in range(2):
                        nc.vector.tensor_copy(raw[:, h2 * 448:(h2 + 1) * 448],
                                              psq[ob * 2 + h2][:])
                    _rope(nc, ev, raw, Ctab, Stab, QT[ob])

                # og1 Q projection (fp8 DR), tiles 4..7
                psq1 = [ps1.tile([P, 448], F32, tag=f"ps1_{j}",
                                 name=f"psq1_{j}") for j in range(8)]
                for p in range(3):
                    for i in range(NI // 2):
                        for ob in range(4):
                            for h2 in range(2):
                                q_mm(1, psq1, p, i, ob, h2)
                for ob in range(4):
                    raw = ev.tile([P, L], BF16, tag="qraw")
                    for h2 in range(2):
                        nc.vector.tensor_copy(raw[:, h2 * 448:(h2 + 1) * 448],
                                              psq1[ob * 2 + h2][:])
                    _rope(nc, ev, raw, Ctab, Stab, QT[4 + ob])

                psk = [ps1.tile([P, 448], F32, tag=f"ps1_{j}", name=f"psk{j}")
                       for j in range(4)]
                for p in range(3):
                    xc, wc = PASSES[p]
                    for i in range(NI // 2):
                        for ob in range(2):
                            for h2 in range(2):
                                nc.tensor.matmul(
                                    psk[ob * 2 + h2][:],
                                    WK8[wc][:, 2 * i:2 * i + 2,
                                            ob * P:(ob + 1) * P],
                                    X8[xc][:, 2 * i:2 * i + 2,
                                           h2 * 448:(h2 + 1) * 448],
                                    start=(p == 0 and i == 0),
                                    stop=(p == 2 and i == NI // 2 - 1),
                                    perf_mode=mybir.MatmulPerfMode.DoubleRow)
                for ob in range(2):
                    raw = ev.tile([P, L], BF16, tag="kraw")
                    for h2 in range(2):
                        nc.vector.tensor_copy(raw[:, h2 * 448:(h2 + 1) * 448],
                                              psk[ob * 2 + h2][:])
                    _rope(nc, ev, raw, Ctab, Stab, KT[ob])

                psv = [ps1.tile([P, KVO], F32, tag=f"ps1_{b_}",
                                name=f"psv{b_}") for b_ in range(LB)]
                for p in range(3):
                    xc, wc = PASSES[p]
                    for i in range(NI // 2):
                        for b_ in range(LB):
                            nc.tensor.matmul(
                                psv[b_][:],
                                X8[xc][:, 2 * i:2 * i + 2,
                                       b_ * P:(b_ + 1) * P],
                                WV8[wc][:, 2 * i:2 * i + 2, :],
                                start=(p == 0 and i == 0),
                                stop=(p == 2 and i == NI // 2 - 1),
                                perf_mode=mybir.MatmulPerfMode.DoubleRow)
                            if p == 2 and i == NI // 2 - 1:
                                nc.scalar.copy(Vst[b_][:, 1:9:2, :],
                                               psv[b_][:])


            # ---------- phase 2: attention ------------------------------
            with tc.tile_pool(name="uatt", bufs=6) as upool, \
                 tc.tile_pool(name="rec", bufs=2) as recpool, \
                 tc.tile_pool(name="pss", bufs=2, space="PSUM") as pss, \
                 tc.tile_pool(name="psav", bufs=2, space="PSUM") as pspool:

                psav = {}

                def scores(t, s, bi):
                    no = s * 64
                    blocks = BLOCKS[bi * EXPB:(bi + 1) * EXPB]
                    ng = len(blocks)
                    ps_s = pss.tile([P, EXPB, P], F32, tag="s",
                                    name=f"s{t}_{s}_{bi}")
                    for j, (qb, kb) in enumerate(blocks):
                        nc.tensor.matmul(
                            ps_s[:, j, :],
                            KT[t // 4][no:no + 64, kb * P:(kb + 1) * P],
                            QT[t][no:no + 64, qb * P:(qb + 1) * P],
                            start=True, stop=True,
                            tile_position=(no, 0))
                    U = upool.tile([P, EXPB, P], BF16, tag="u",
                                   name=f"u{t}_{s}_{bi}")
                    nc.scalar.activation(U[:, 0:ng, :], ps_s[:, 0:ng, :],
                                         AF.Exp, scale=0.125)
                    for j, (qb, kb) in enumerate(blocks):
                        if qb == kb:
                            nc.gpsimd.affine_select(
                                out=U[:, j, :], in_=U[:, j, :],
                                compare_op=ALU.is_ge, fill=0.0,
                                base=0, channel_multiplier=-1,
                                pattern=[[1, P]])
                    return U, blocks

                def avs(t, s, bi, U, blocks):
                    kv = 2 * (t // 4) + s
                    if (t, s) not in psav:
                        psav[(t, s)] = pspool.tile([P, L], F32, tag="av",
                                                   name=f"av{t}_{s}")
                    for j, (qb, kb) in enumerate(blocks):
                        nc.tensor.matmul(
                            psav[(t, s)][:, qb * P:(qb + 1) * P],
                            _sb(Vst[kb], kv * P + (1 - s) * 64,
                                [[576, P], [1, P]]),
                            U[:, j, :],
                            start=(kb == 0), stop=(kb == qb))
                    if bi == NBATCH - 1:
                        epilogue(t, s)

                def epilogue(t, s):
                    no, so = s * 64, (1 - s) * 64
                    rec = recpool.tile([P, L], F32, tag="rec",
                                       name=f"rec{t}_{s}")
                    nc.vector.reciprocal(rec[so:so + 64, :],
                                         psav[(t, s)][so:so + 64, :])
                    rec2 = recpool.tile([P, L], F32, tag="rec2",
                                        name=f"rec2{t}_{s}")
                    nc.sync.dma_start(rec2[no:no + 64, :],
                                      rec[so:so + 64, :])
                    nc.vector.tensor_mul(AT[t][no:no + 64, :],
                                         psav[(t, s)][no:no + 64, :],
                                         rec2[no:no + 64, :])

                    def split(t=t, no=no):
                        with nc.allow_low_precision(reason="3-term fp8 "
                                                    "split beats bf16"):
                            nc.vector.tensor_scalar_mul(
                                AT8["h"][no:no + 64, t, :],
                                AT[t][no:no + 64, :], 16.0)
                            nc.vector.scalar_tensor_tensor(
                                AT8["l"][no:no + 64, t, :],
                                AT[t][no:no + 64, :], 16.0,
                                AT8["h"][no:no + 64, t, :],
                                op0=ALU.mult, op1=ALU.subtract)
                    if t < 7:
                        split()   # mid-stream: DVE slack, off critical path
                    else:
                        late_splits.append(split)  # keep t7 epilogues tight

                # flat software pipeline: AVs lag scores by 2 batch-pairs
                fifo = []
                late_splits = []
                for t in range(8):
                    for bi in range(NBATCH):
                        for s in range(2):
                            fifo.append((t, s, bi) + scores(t, s, bi))
                        while len(fifo) > 4:
                            avs(*fifo.pop(0))
                    # drain stream A's tail so its epilogue overlaps the
                    # next tile's scores; B's last AV follows one pair in
                    while len(fifo) > (1 if t < 7 else 0):
                        avs(*fifo.pop(0))
                for sp in late_splits:
                    sp()

        # ---------------- phase 3: out projection ------------------------
        with tc.tile_pool(name="osb", bufs=2) as osb, \
             tc.tile_pool(name="pso", bufs=1, space="PSUM") as pso:
            def p3_mm(ps, oc, b_, p, dr):
                ac, wc = PASSES[p]
                nc.tensor.matmul(
                    ps[:], AT8[ac][:, 2 * dr:2 * dr + 2,
                                   b_ * P:(b_ + 1) * P],
                    WO8[wc][:, 2 * dr:2 * dr + 2,
                            oc * 512:(oc + 1) * 512],
                    start=(p == 0 and dr == 0), stop=(p == 2 and dr == 3),
                    perf_mode=mybir.MatmulPerfMode.DoubleRow)

            for oc in range(4):
                ob_t = osb.tile([P, LB, 512], BF16, tag="ot", name=f"ot{oc}")
                # 4 psum banks (the ones freed before the last epilogues);
                # for oc 0 defer ic=7 (needs the last head's AT) past a full
                # wave of ic 0..6 so the PE never waits on the epilogue tail
                for w0, wn in ((0, 4), (4, 3)):
                    ps_w = []
                    for b_ in range(w0, w0 + wn):
                        ps = pso.tile([P, 512], F32, tag=f"pso{b_ % 4}",
                                      name=f"pso{oc}_{b_}")
                        ps_w.append(ps)
                        for p in range(3):
                            for dr in range(4 if oc else 3):
                                p3_mm(ps, oc, b_, p, dr)
                        if oc != 0:
                            if oc == 3 and b_ >= 4:
                                nc.vector.tensor_copy(ob_t[:, b_, :], ps[:])
                            else:
                                nc.scalar.copy(ob_t[:, b_, :], ps[:])
                    if oc == 0:
                        for j, b_ in enumerate(range(w0, w0 + wn)):
                            for p in range(3):
                                p3_mm(ps_w[j], oc, b_, p, 3)
                            nc.scalar.copy(ob_t[:, b_, :], ps_w[j][:])
                if oc < 3:
                    nc.sync.dma_start(
                        _ap3(y, oc * 512, [[D, P], [P * D, LB], [1, 512]]),
                        ob_t[:])
                else:
                    nc.sync.dma_start(
                        _ap3(y, oc * 512, [[D, P], [P * D, 4], [1, 512]]),
                        ob_t[:, 0:4, :])
                    nc.sync.dma_start(
                        _ap3(y, oc * 512 + 4 * P * D,
                             [[D, P], [P * D, 3], [1, 512]]),
                        ob_t[:, 4:7, :])


# ---------------------------------------------------------------- host side
ROPE_BASE = 10000.0
_ROPE_PERM = np.concatenate([
    np.arange(0, 32, 2), np.arange(1, 32, 2),
    np.arange(32, 64, 2), np.arange(33, 64, 2)])
# local q head at (tile t, slot s) = _LOCAL_HEADS[2*t + s]
_LOCAL_HEADS = [8 * (t // 4) + 4 * s + t % 4 for t in range(8) for s in range(2)]


F8NP = (ml_dtypes.float8_e4m3fn if hasattr(ml_dtypes, "float8_e4m3fn")
        else ml_dtypes.float8_e4m3)


def _split8(a, scale):
    """hi/lo fp8 split at a single scale: a ~= (hi + lo)/scale."""
    hi = np.asarray(a * scale, dtype=F8NP)
    lo = np.asarray(a * scale - hi.astype(np.float32), dtype=F8NP)
    return np.ascontiguousarray(hi), np.ascontiguousarray(lo)


def _cos_sin_tables(temporal_pos, structural_pos):
    inv = (1.0 / ROPE_BASE) ** (np.arange(16, dtype=np.float64) / 16.0)
    tabs = {}
    for name, pos in (("t", temporal_pos), ("s", structural_pos)):
        ang = np.outer(inv, np.asarray(pos, dtype=np.float64))  # [16, L]
        tabs[name] = (np.cos(ang), np.sin(ang))
    ct, st = tabs["t"]
    cs, ss = tabs["s"]
    # 1/(SX*SW) folds the fp8 psum scale out of the roped q/k
    C64 = np.concatenate([ct, ct, cs, cs], axis=0) / (SX * SW)
    S64 = np.concatenate([-st, st, -ss, ss], axis=0) / (SX * SW)
    C = np.tile(C64, (2, 1)).astype(ml_dtypes.bfloat16)
    S = np.tile(S64, (2, 1)).astype(ml_dtypes.bfloat16)
    return np.ascontiguousarray(C), np.ascontiguousarray(S)


def make_in_maps(x, wq, wk, wv, wo, temporal_pos, structural_pos):
    bf = ml_dtypes.bfloat16
    x = np.asarray(x, dtype=np.float32)
    wq4 = np.asarray(wq, dtype=np.float32).reshape(HQ, HD, D)
    wk4 = np.asarray(wk, dtype=np.float32).reshape(HKV, HD, D)
    wv4 = np.asarray(wv, dtype=np.float32).reshape(HKV, HD, D)
    woT = np.asarray(wo, dtype=np.float32).T  # [D(in head dims), D(out)]
    C, S = _cos_sin_tables(temporal_pos, structural_pos)
    SWAP = np.zeros((P, P), dtype=ml_dtypes.bfloat16)
    for i in range(P):
        SWAP[i, i ^ 64] = 1.0

    in_maps = []
    for c in range(NCORES):
        b, g = divmod(c, 2)
        heads = [16 * g + h for h in _LOCAL_HEADS]
        wq_g = wq4[heads][:, _ROPE_PERM, :].reshape(GO, D)
        wk_g = wk4[4 * g:4 * g + 4][:, _ROPE_PERM, :].reshape(KVO, D)
        wv_g = wv4[4 * g:4 * g + 4].reshape(KVO, D)
        woT_g = np.concatenate([woT[64 * h:64 * h + 64, :] for h in heads])
        m = {"woT": np.ascontiguousarray(woT_g).astype(bf), "C": C, "S": S,
             "SW": SWAP}
        m["xTh"], m["xTl"] = _split8(np.ascontiguousarray(x[b].T), SX)
        m["wqTh"], m["wqTl"] = _split8(np.ascontiguousarray(wq_g.T), SW)
        m["wkTh"], m["wkTl"] = _split8(np.ascontiguousarray(wk_g.T), SW)
        m["wvTh"], m["wvTl"] = _split8(np.ascontiguousarray(wv_g.T), SW)
        in_maps.append(m)
    return in_maps


def kernel(x, wq, wk, wv, wo, temporal_pos, structural_pos, _trace=False):
    nc = build_nc()
    in_maps = make_in_maps(x, wq, wk, wv, wo, temporal_pos, structural_pos)
    res = bass_utils.run_bass_kernel_spmd(
        nc, in_maps, core_ids=list(range(NCORES)), trace=_trace)
    out = np.stack([
        np.asarray(res.results[2 * b]["y"], dtype=np.float32)
        + np.asarray(res.results[2 * b + 1]["y"], dtype=np.float32)
        for b in range(B)]) / (16.0 * SW)
    kernel.last_result = res
    return out


# revision 57
# speedup vs baseline: 1.0090x; 1.0090x over previous
"""GQA attention with 2D RoPE on 8 TRN2 NeuronCores.

Sharding: batch data-parallel x4  X  head-group tensor-parallel x2.
Core c handles batch b=c//2 and head group g=c%2 (16 Q heads, 4 KV heads).
Each core writes its PARTIAL out-projection y_g = A_g @ woT_g  [896, 2048]
(bf16); the host sums the two partials per batch (no device collective).

All weights/activations enter as bf16 (host-cast).  RoPE cos/sin tables are
host-computed.  Q heads are host-permuted so the 4 Q heads of each KV head
live in the same partition half as that KV head (no K duplication): QT tile
t, slot s  <->  local q head  8*(t//4) + 4*s + t%4, kv head 2*(t//4)+s.

The Q/K/V projections run as 3-term fp8 splits (x = xh+xl, w = wh+wl at a
shared scale, dropping the xl*wl term) with DoubleRow perf mode, which this
PE executes at 4x bf16 throughput; accuracy beats plain bf16.  The 1/512
psum scale folds into the host RoPE tables (q, k) and the Vst ones columns
(denominator), so no extra scaling ops exist on device.

Attention uses 128-wide q blocks, 28 causal (qb, kb) blocks per head in
qb-major order, batched 8-at-a-time so exp runs as 4 big Activation
instructions per head (the Activation engine is the attention bottleneck).
The two slots of each QT tile run as two interleaved streams in a flat
software pipeline where AVs lag scores by 2 batch-pairs; each tile's tail
is flushed early so softmax epilogues overlap the next tile's scores.

V staging per key block kb: Vst[kb] [128 keys, 9, 64] = [1|V0|1|V1|...|V3|1]
(64-col groups).  AV lhsT for kv head k, slot s = cols [128k + 64*(1-s),
+128) which is [V_k|1] for s=0 (numerators rows 0:64, denominator sums rows
64:128) and [1|V_k] for s=1.
"""

import numpy as np
import ml_dtypes

import concourse.bass as bass
import concourse.tile as tile
import concourse.mybir as mybir
from concourse import bacc
from concourse import bass_utils

F32 = mybir.dt.float32
BF16 = mybir.dt.bfloat16
F8 = mybir.dt.float8e4
AF = mybir.ActivationFunctionType
ALU = mybir.AluOpType

B, L, D = 4, 896, 2048
HQ, HKV, HD = 32, 8, 64
NCORES = 8
GO = D // 2          # 1024 q-out dims per core
KVO = HKV * HD // 2  # 256 kv-out dims per core
P = 128
NI = D // P          # 16 contraction chunks
LB = L // P          # 7 key/q blocks

# causal blocks per head, qb-major: [(qb, kb)], kb <= qb
BLOCKS = [(qb, kb) for qb in range(LB) for kb in range(qb + 1)]
SX, SW = 8.0, 64.0   # fp8 operand scales (psum carries SX*SW = 512x)
EXPB = 8             # blocks per exp batch
NBATCH = (len(BLOCKS) + EXPB - 1) // EXPB   # 4 batches: 8,8,8,4

_NC_CACHE = {}


def _ap3(dram_ap, off, ap):
    return bass.AP(tensor=dram_ap.tensor, offset=dram_ap.offset + off, ap=ap)


def _sb(tile_ap, off, ap):
    """Manual sub-AP of an SBUF tile."""
    return bass.AP(tensor=tile_ap.tensor, offset=tile_ap.offset + off, ap=ap)


def build_nc():
    if "nc" in _NC_CACHE:
        return _NC_CACHE["nc"]
    nc = bacc.Bacc("TRN2", target_bir_lowering=False, debug=False,
                   num_devices=NCORES)
    ins = {"woT": nc.dram_tensor("woT", [GO, D], BF16,
                                 kind="ExternalInput").ap(),
           "C": nc.dram_tensor("C", [P, L], BF16, kind="ExternalInput").ap(),
           "S": nc.dram_tensor("S", [P, L], BF16, kind="ExternalInput").ap(),
           "SW": nc.dram_tensor("SW", [P, P], BF16,
                                kind="ExternalInput").ap()}
    for nm, shp in (("xT", [D, L]), ("wqT", [D, GO]), ("wkT", [D, KVO]),
                    ("wvT", [D, KVO])):
        for c in "hl":
            ins[nm + c] = nc.dram_tensor(nm + c, shp, F8,
                                         kind="ExternalInput").ap()
    y = nc.dram_tensor("y", [L, D], BF16, kind="ExternalOutput").ap()
    with tile.TileContext(nc) as tc:
        _build_kernel(nc, tc, ins, y)
    nc.compile()
    _NC_CACHE["nc"] = nc
    return nc


def _rope(nc, pool, raw, C, S, out):
    """out = raw*C + shuffle16(raw)*S   (all [128, L] bf16)."""
    shuf = pool.tile([P, L], BF16, tag="rope_shuf")
    mask = [(p ^ 16) for p in range(32)]
    nc.vector.stream_shuffle(shuf[:], raw[:], mask)
    m1 = pool.tile([P, L], BF16, tag="rope_m1", bufs=1)
    nc.vector.tensor_mul(m1[:], raw[:], C[:])
    m2 = pool.tile([P, L], BF16, tag="rope_m2", bufs=1)
    nc.vector.tensor_mul(m2[:], shuf[:], S[:])
    nc.vector.tensor_add(out[:], m1[:], m2[:])


def _build_kernel(nc, tc, ins, y):
    import contextlib
    ctx = contextlib.ExitStack()
    with ctx:
        const = ctx.enter_context(tc.tile_pool(name="const", bufs=1))
        qt_pool = ctx.enter_context(tc.tile_pool(name="qt", bufs=1))
        kt_pool = ctx.enter_context(tc.tile_pool(name="kt", bufs=1))
        v_pool = ctx.enter_context(tc.tile_pool(name="vx", bufs=1))
        at_pool = ctx.enter_context(tc.tile_pool(name="at", bufs=1))
        wo_pool = ctx.enter_context(tc.tile_pool(name="wo", bufs=1))
        QT = [qt_pool.tile([P, L], BF16, tag=f"qt{t}", name=f"qt{t}")
              for t in range(8)]
        KT = [kt_pool.tile([P, L], BF16, tag=f"kt{i}", name=f"kt{i}")
              for i in range(2)]
        # Vst[kb]: [1|V0|1|V1|1|V2|1|V3|1], 9 64-col groups
        Vst = [v_pool.tile([P, 9, 64], BF16, tag=f"v{b_}", name=f"v{b_}")
               for b_ in range(LB)]
        AT = [at_pool.tile([P, L], BF16, tag=f"at{t}", name=f"at{t}")
              for t in range(8)]
        AT8 = {c: at_pool.tile([P, 8, L], F8, tag=f"at8{c}", name=f"at8{c}")
               for c in "hl"}
        WO8 = {c: wo_pool.tile([P, 8, D], F8, tag=f"wo8{c}", name=f"wo8{c}")
               for c in "hl"}
        Ctab = const.tile([P, L], BF16, tag="Ctab", name="Ctab")
        Stab = const.tile([P, L], BF16, tag="Stab", name="Stab")
        dum_src = const.tile([P, P], BF16, tag="dum_src", name="dum_src")
        nc.vector.memset(dum_src[:], 0.0)
        SWt = const.tile([P, P], BF16, tag="SWt", name="SWt")
        for b_ in range(LB):
            for j in range(5):
                nc.vector.memset(Vst[b_][:, 2 * j, :], SX * SW)

        with tc.tile_pool(name="xt", bufs=1) as xt_pool, \
             tc.tile_pool(name="wst", bufs=1) as wst, \
             tc.tile_pool(name="ev", bufs=2) as ev:
            X8 = {c: xt_pool.tile([P, NI, L], F8, tag=f"x{c}", name=f"x{c}")
                  for c in "hl"}
            WV8 = {c: wst.tile([P, NI, KVO], F8, tag=f"wv{c}", name=f"wv{c}")
                   for c in "hl"}
            WK8 = {c: wst.tile([P, NI, KVO], F8, tag=f"wk{c}", name=f"wk{c}")
                   for c in "hl"}
            WQ8 = [{c: wst.tile([P, NI, 512], F8, tag=f"wq{og}{c}",
                               name=f"wq{og}{c}") for c in "hl"}
                   for og in range(2)]

            def ld_x(c, c0, cn):
                nc.sync.dma_start(
                    X8[c][:, c0:c0 + cn, :],
                    _ap3(ins["xT" + c], c0 * P * L,
                         [[L, P], [P * L, cn], [1, L]]))

            def ld_wq(og, c, c0, cn):
                nc.sync.dma_start(
                    WQ8[og][c][:, c0:c0 + cn, :],
                    _ap3(ins["wqT" + c], og * 512 + c0 * P * GO,
                         [[GO, P], [P * GO, cn], [1, 512]]))

            def ld_wkv(name, dst, c, c0, cn):
                nc.sync.dma_start(
                    dst[c][:, c0:c0 + cn, :],
                    _ap3(ins[name + c], c0 * P * KVO,
                         [[KVO, P], [P * KVO, cn], [1, KVO]]))

            # strict consumption order: Q0 pass0 (wq0h+xh), pass1 (wq0l),
            # pass2 (xl), og1 weights, rope tables, K, V, wo
            for c0 in range(0, 16, 4):
                ld_wq(0, "h", c0, 4); ld_x("h", c0, 4); ld_wq(0, "l", c0, 4)
            for c0 in range(0, 16, 4):
                ld_x("l", c0, 4)
            ld_wq(1, "h", 0, 4); ld_wq(1, "h", 4, 4)
            ld_wq(1, "h", 8, 4); ld_wq(1, "h", 12, 4)
            ld_wq(1, "l", 0, 8); ld_wq(1, "l", 8, 8)
            nc.sync.dma_start(Ctab[:], ins["C"][:, :])
            nc.sync.dma_start(Stab[:], ins["S"][:, :])
            nc.sync.dma_start(SWt[:], ins["SW"][:, :])
            ld_wkv("wkT", WK8, "h", 0, 16); ld_wkv("wkT", WK8, "l", 0, 16)
            ld_wkv("wvT", WV8, "h", 0, 16); ld_wkv("wvT", WV8, "l", 0, 16)
            for c in "hl":
                for h in range(2):
                    nc.sync.dma_start(
                        WO8[c][:, 4 * h:4 * h + 4, :],
                        _ap3(ins["woT" + c], 4 * h * P * D,
                             [[D, P], [P * D, 4], [1, D]]))

            # ---------- phase 1: Q og0, K, V projections -----------------
            with tc.tile_pool(name="ps1", bufs=1, space="PSUM") as ps1:
                dumps = ps1.tile([P, P], F32, tag="ps1_7", name="dumps")

                def dummies(n):
                    # keep the PE continuously busy across a known stall so
                    # the p-state never drops (each is [128,128], ~53 ns)
                    for _ in range(n):
                        nc.tensor.matmul(dumps[:], dum_src[:], dum_src[:],
                                         start=True, stop=True)

                dummies(38)
                # 3-term fp8 split: xh@wh + xh@wl + xl@wh, DoubleRow pairs
                # of 128-chunks (8 dr-chunks of 256 contraction each)
                PASSES = (("h", "h"), ("h", "l"), ("l", "h"))
                psq = [ps1.tile([P, 448], F32, tag=f"ps1_{j}",
                                name=f"psq0_{j}") for j in range(8)]

                def q_mm(og, psum, p, i, ob, h2):
                    xc, wc = PASSES[p]
                    nc.tensor.matmul(
                        psum[ob * 2 + h2][:],
                        WQ8[og][wc][:, 2 * i:2 * i + 2, ob * P:(ob + 1) * P],
                        X8[xc][:, 2 * i:2 * i + 2,
                               h2 * 448:(h2 + 1) * 448],
                        start=(p == 0 and i == 0),
                        stop=(p == 2 and i == NI // 2 - 1),
                        perf_mode=mybir.MatmulPerfMode.DoubleRow)

                # interleave passes 0/1 per chunk: pass0 alone is gated by
                # the wq0h+xh DMA stream; pass1 reuses xh so it fills the gaps
                for i in range(NI // 2):
                    for p in range(2):
                        for ob in range(4):
                            for h2 in range(2):
                                q_mm(0, psq, p, i, ob, h2)
                for i in range(NI // 2):
                    for ob in range(4):
                        for h2 in range(2):
                            q_mm(0, psq, 2, i, ob, h2)
                for ob in range(4):
                    raw = ev.tile([P, L], BF16, tag="qraw")
                    for h2 in range(2):
                        nc.vector.tensor_copy(raw[:, h2 * 448:(h2 + 1) * 448],
                                              psq[ob * 2 + h2][:])
                    _rope(nc, ev, raw, Ctab, Stab, QT[ob])

                # og1 Q projection (fp8 DR), tiles 4..7
                psq1 = [ps1.tile([P, 448], F32, tag=f"ps1_{j}",
                                 name=f"psq1_{j}") for j in range(8)]
                for p in range(3):
                    for i in range(NI // 2):
                        for ob in range(4):
                            for h2 in range(2):
                                q_mm(1, psq1, p, i, ob, h2)
                for ob in range(4):
                    raw = ev.tile([P, L], BF16, tag="qraw")
                    for h2 in range(2):
                        nc.vector.tensor_copy(raw[:, h2 * 448:(h2 + 1) * 448],
                                              psq1[ob * 2 + h2][:])
                    _rope(nc, ev, raw, Ctab, Stab, QT[4 + ob])

                psk = [ps1.tile([P, 448], F32, tag=f"ps1_{j}", name=f"psk{j}")
                       for j in range(4)]
                for p in range(3):
                    xc, wc = PASSES[p]
                    for i in range(NI // 2):
                        for ob in range(2):
                            for h2 in range(2):
                                nc.tensor.matmul(
                                    psk[ob * 2 + h2][:],
                                    WK8[wc][:, 2 * i:2 * i + 2,
                                            ob * P:(ob + 1) * P],
                                    X8[xc][:, 2 * i:2 * i + 2,
                                           h2 * 448:(h2 + 1) * 448],
                                    start=(p == 0 and i == 0),
                                    stop=(p == 2 and i == NI // 2 - 1),
                                    perf_mode=mybir.MatmulPerfMode.DoubleRow)
                for ob in range(2):
                    raw = ev.tile([P, L], BF16, tag="kraw")
                    for h2 in range(2):
                        nc.vector.tensor_copy(raw[:, h2 * 448:(h2 + 1) * 448],
                                              psk[ob * 2 + h2][:])
                    _rope(nc, ev, raw, Ctab, Stab, KT[ob])

                psv = [ps1.tile([P, KVO], F32, tag=f"ps1_{b_}",
                                name=f"psv{b_}") for b_ in range(LB)]
                for p in range(3):
                    xc, wc = PASSES[p]
                    for i in range(NI // 2):
                        for b_ in range(LB):
                            nc.tensor.matmul(
                                psv[b_][:],
                                X8[xc][:, 2 * i:2 * i + 2,
                                       b_ * P:(b_ + 1) * P],
                                WV8[wc][:, 2 * i:2 * i + 2, :],
                                start=(p == 0 and i == 0),
                                stop=(p == 2 and i == NI // 2 - 1),
                                perf_mode=mybir.MatmulPerfMode.DoubleRow)
                            if p == 2 and i == NI // 2 - 1:
                                nc.scalar.copy(Vst[b_][:, 1:9:2, :],
                                               psv[b_][:])


            # ---------- phase 2: attention ------------------------------
            with tc.tile_pool(name="uatt", bufs=6) as upool, \
                 tc.tile_pool(name="rec", bufs=2) as recpool, \
                 tc.tile_pool(name="pss", bufs=2, space="PSUM") as pss, \
                 tc.tile_pool(name="psav", bufs=2, space="PSUM") as pspool:

                psav = {}

                def scores(t, s, bi):
                    no = s * 64
                    blocks = BLOCKS[bi * EXPB:(bi + 1) * EXPB]
                    ng = len(blocks)
                    ps_s = pss.tile([P, EXPB, P], F32, tag="s",
                                    name=f"s{t}_{s}_{bi}")
                    for j, (qb, kb) in enumerate(blocks):
                        nc.tensor.matmul(
                            ps_s[:, j, :],
                            KT[t // 4][no:no + 64, kb * P:(kb + 1) * P],
                            QT[t][no:no + 64, qb * P:(qb + 1) * P],
                            start=True, stop=True,
                            tile_position=(no, 0))
                    U = upool.tile([P, EXPB, P], BF16, tag="u",
                                   name=f"u{t}_{s}_{bi}")
                    nc.scalar.activation(U[:, 0:ng, :], ps_s[:, 0:ng, :],
                                         AF.Exp, scale=0.125)
                    for j, (qb, kb) in enumerate(blocks):
                        if qb == kb:
                            nc.gpsimd.affine_select(
                                out=U[:, j, :], in_=U[:, j, :],
                                compare_op=ALU.is_ge, fill=0.0,
                                base=0, channel_multiplier=-1,
                                pattern=[[1, P]])
                    return U, blocks

                def avs(t, s, bi, U, blocks):
                    kv = 2 * (t // 4) + s
                    if (t, s) not in psav:
                        psav[(t, s)] = pspool.tile([P, L], F32, tag="av",
                                                   name=f"av{t}_{s}")
                    for j, (qb, kb) in enumerate(blocks):
                        nc.tensor.matmul(
                            psav[(t, s)][:, qb * P:(qb + 1) * P],
                            _sb(Vst[kb], kv * P + (1 - s) * 64,
                                [[576, P], [1, P]]),
                            U[:, j, :],
                            start=(kb == 0), stop=(kb == qb))
                    if bi == NBATCH - 1:
                        epilogue(t, s)

                def epilogue(t, s):
                    no, so = s * 64, (1 - s) * 64
                    rec = recpool.tile([P, L], F32, tag="rec",
                                       name=f"rec{t}_{s}")
                    nc.vector.reciprocal(rec[so:so + 64, :],
                                         psav[(t, s)][so:so + 64, :])
                    rec2 = recpool.tile([P, L], F32, tag="rec2",
                                        name=f"rec2{t}_{s}")
                    nc.sync.dma_start(rec2[no:no + 64, :],
                                      rec[so:so + 64, :])
                    nc.vector.tensor_mul(AT[t][no:no + 64, :],
                                         psav[(t, s)][no:no + 64, :],
                                         rec2[no:no + 64, :])

                    def split(t=t, no=no):
                        with nc.allow_low_precision(reason="3-term fp8 "
                                                    "split beats bf16"):
                            nc.vector.tensor_scalar_mul(
                                AT8["h"][no:no + 64, t, :],
                                AT[t][no:no + 64, :], 16.0)
                            nc.vector.scalar_tensor_tensor(
                                AT8["l"][no:no + 64, t, :],
                                AT[t][no:no + 64, :], 16.0,
                                AT8["h"][no:no + 64, t, :],
                                op0=ALU.mult, op1=ALU.subtract)
                    if t < 7:
                        split()   # mid-stream: DVE slack, off critical path
                    else:
                        late_splits.append(split)  # keep t7 epilogues tight

                # flat software pipeline: AVs lag scores by 2 batch-pairs
                fifo = []
                late_splits = []
                for t in range(8):
                    for bi in range(NBATCH):
                        for s in range(2):
                            fifo.append((t, s, bi) + scores(t, s, bi))
                        while len(fifo) > 4:
                            avs(*fifo.pop(0))
                    # drain stream A's tail so its epilogue overlaps the
                    # next tile's scores; B's last AV follows one pair in
                    while len(fifo) > 0:
                        avs(*fifo.pop(0))
                for sp in late_splits:
                    sp()

        # ---------------- phase 3: out projection ------------------------
        with tc.tile_pool(name="osb", bufs=2) as osb, \
             tc.tile_pool(name="pso", bufs=1, space="PSUM") as pso:
            def p3_mm(ps, oc, b_, p, dr):
                ac, wc = PASSES[p]
                nc.tensor.matmul(
                    ps[:], AT8[ac][:, 2 * dr:2 * dr + 2,
                                   b_ * P:(b_ + 1) * P],
                    WO8[wc][:, 2 * dr:2 * dr + 2,
                            oc * 512:(oc + 1) * 512],
                    start=(p == 0 and dr == 0), stop=(p == 2 and dr == 3),
                    perf_mode=mybir.MatmulPerfMode.DoubleRow)

            for oc in range(4):
                ob_t = osb.tile([P, LB, 512], BF16, tag="ot", name=f"ot{oc}")
                # 4 psum banks (the ones freed before the last epilogues);
                # for oc 0 defer ic=7 (needs the last head's AT) past a full
                # wave of ic 0..6 so the PE never waits on the epilogue tail
                for w0, wn in ((0, 4), (4, 3)):
                    ps_w = []
                    for b_ in range(w0, w0 + wn):
                        ps = pso.tile([P, 512], F32, tag=f"pso{b_ % 4}",
                                      name=f"pso{oc}_{b_}")
                        ps_w.append(ps)
                        for p in range(3):
                            for dr in range(4 if oc else 3):
                                p3_mm(ps, oc, b_, p, dr)
                        if oc != 0:
                            if oc == 3 and b_ >= 4:
                                nc.vector.tensor_copy(ob_t[:, b_, :], ps[:])
                            else:
                                nc.scalar.copy(ob_t[:, b_, :], ps[:])
                    if oc == 0:
                        for j, b_ in enumerate(range(w0, w0 + wn)):
                            for p in range(3):
                                p3_mm(ps_w[j], oc, b_, p, 3)
                            nc.scalar.copy(ob_t[:, b_, :], ps_w[j][:])
                if oc < 3:
                    nc.sync.dma_start(
                        _ap3(y, oc * 512, [[D, P], [P * D, LB], [1, 512]]),
                        ob_t[:])
                else:
                    nc.sync.dma_start(
                        _ap3(y, oc * 512, [[D, P], [P * D, 4], [1, 512]]),
                        ob_t[:, 0:4, :])
                    nc.sync.dma_start(
                        _ap3(y, oc * 512 + 4 * P * D,
                             [[D, P], [P * D, 3], [1, 512]]),
                        ob_t[:, 4:7, :])


# ---------------------------------------------------------------- host side
ROPE_BASE = 10000.0
_ROPE_PERM = np.concatenate([
    np.arange(0, 32, 2), np.arange(1, 32, 2),
    np.arange(32, 64, 2), np.arange(33, 64, 2)])
# local q head at (tile t, slot s) = _LOCAL_HEADS[2*t + s]
_LOCAL_HEADS = [8 * (t // 4) + 4 * s + t % 4 for t in range(8) for s in range(2)]


F8NP = (ml_dtypes.float8_e4m3fn if hasattr(ml_dtypes, "float8_e4m3fn")
        else ml_dtypes.float8_e4m3)


def _split8(a, scale):
    """hi/lo fp8 split at a single scale: a ~= (hi + lo)/scale."""
    hi = np.asarray(a * scale, dtype=F8NP)
    lo = np.asarray(a * scale - hi.astype(np.float32), dtype=F8NP)
    return np.ascontiguousarray(hi), np.ascontiguousarray(lo)


def _cos_sin_tables(temporal_pos, structural_pos):
    inv = (1.0 / ROPE_BASE) ** (np.arange(16, dtype=np.float64) / 16.0)
    tabs = {}
    for name, pos in (("t", temporal_pos), ("s", structural_pos)):
        ang = np.outer(inv, np.asarray(pos, dtype=np.float64))  # [16, L]
        tabs[name] = (np.cos(ang), np.sin(ang))
    ct, st = tabs["t"]
    cs, ss = tabs["s"]
    # 1/(SX*SW) folds the fp8 psum scale out of the roped q/k
    C64 = np.concatenate([ct, ct, cs, cs], axis=0) / (SX * SW)
    S64 = np.concatenate([-st, st, -ss, ss], axis=0) / (SX * SW)
    C = np.tile(C64, (2, 1)).astype(ml_dtypes.bfloat16)
    S = np.tile(S64, (2, 1)).astype(ml_dtypes.bfloat16)
    return np.ascontiguousarray(C), np.ascontiguousarray(S)


def make_in_maps(x, wq, wk, wv, wo, temporal_pos, structural_pos):
    bf = ml_dtypes.bfloat16
    x = np.asarray(x, dtype=np.float32)
    wq4 = np.asarray(wq, dtype=np.float32).reshape(HQ, HD, D)
    wk4 = np.asarray(wk, dtype=np.float32).reshape(HKV, HD, D)
    wv4 = np.asarray(wv, dtype=np.float32).reshape(HKV, HD, D)
    woT = np.asarray(wo, dtype=np.float32).T  # [D(in head dims), D(out)]
    C, S = _cos_sin_tables(temporal_pos, structural_pos)
    SWAP = np.zeros((P, P), dtype=ml_dtypes.bfloat16)
    for i in range(P):
        SWAP[i, i ^ 64] = 1.0

    in_maps = []
    for c in range(NCORES):
        b, g = divmod(c, 2)
        heads = [16 * g + h for h in _LOCAL_HEADS]
        wq_g = wq4[heads][:, _ROPE_PERM, :].reshape(GO, D)
        wk_g = wk4[4 * g:4 * g + 4][:, _ROPE_PERM, :].reshape(KVO, D)
        wv_g = wv4[4 * g:4 * g + 4].reshape(KVO, D)
        woT_g = np.concatenate([woT[64 * h:64 * h + 64, :] for h in heads])
        m = {"woT": np.ascontiguousarray(woT_g).astype(bf), "C": C, "S": S,
             "SW": SWAP}
        m["xTh"], m["xTl"] = _split8(np.ascontiguousarray(x[b].T), SX)
        m["wqTh"], m["wqTl"] = _split8(np.ascontiguousarray(wq_g.T), SW)
        m["wkTh"], m["wkTl"] = _split8(np.ascontiguousarray(wk_g.T), SW)
        m["wvTh"], m["wvTl"] = _split8(np.ascontiguousarray(wv_g.T), SW)
        in_maps.append(m)
    return in_maps


def kernel(x, wq, wk, wv, wo, temporal_pos, structural_pos, _trace=False):
    nc = build_nc()
    in_maps = make_in_maps(x, wq, wk, wv, wo, temporal_pos, structural_pos)
    res = bass_utils.run_bass_kernel_spmd(
        nc, in_maps, core_ids=list(range(NCORES)), trace=_trace)
    out = np.stack([
        np.asarray(res.results[2 * b]["y"], dtype=np.float32)
        + np.asarray(res.results[2 * b + 1]["y"], dtype=np.float32)
        for b in range(B)]) / (16.0 * SW)
    kernel.last_result = res
    return out
